# revision 26
# baseline (speedup 1.0000x reference)
"""Trainium2 kernel for nn_DecodePredictions (YOLO-style decode + greedy NMS).

Strategy:
  - The memory-bound bulk (reading the [1,512,512,100] f32 input, per-cell
    class-max over 90 classes, per-anchor score = cls_max * objectness) runs
    on 8 NeuronCores, sharded over the 512 grid rows (64 rows per core).
  - Each core streams its 13.1 MB slice through SBUF with big contiguous
    DMAs and produces a [2, 32768] f32 score plane (anchor-major).
  - The greedy NMS only ever touches the top ~30 candidates (the reference
    loop locks onto a zero-area box and repeats it), so it runs on host in
    float32 numpy, replicating the reference op-for-op (verified bitwise
    identical against the JAX reference).
"""

import numpy as np

G = 512
C = 100
NCORES = 8
ROWS = G // NCORES            # 64 grid rows per core
CELLS = ROWS * G              # 32768 cells per core
K = 32                        # cells per SBUF partition per tile
NT = CELLS // (128 * K)       # 8 tiles per core
N_CLS = 90

STRIDE = np.float32(16.0)
INPUT_SIZE = np.float32(8192.0)
IOU_THR = np.float32(0.5)
SCORE_THR = np.float32(0.6)
MAX_OUT = 100

_CACHE = {}


def _legalize_multi_waits(nc):
    """This toolchain's walrus rejects any instruction whose sync_info
    carries 2+ waits ("Too many sync wait commands" — the TPB EVENTS slot
    holds exactly one wait). Tile emits multi-wait instructions expecting
    the backend to split them, so do the split here: hoist all but the
    last wait onto single-wait engine NoOps inserted just before the
    instruction (same engine stream => same ordering guarantee)."""
    import concourse.mybir as mybir

    ctr = 0
    for bb in nc.main_func.blocks:
        out = []
        for ins in bb.instructions:
            si = getattr(ins, "sync_info", None)
            waits = list(si.on_wait) if (si is not None and si.on_wait) else []
            if len(waits) > 1:
                for w in waits[:-1]:
                    ctr += 1
                    nop = mybir.InstNoOp(
                        name=f"I-waitsplit-{ctr}", engine=ins.engine
                    )
                    nop.sync_info = mybir.SyncInfo(on_wait=[w], on_update=[])
                    out.append(nop)
                ins.sync_info = mybir.SyncInfo(
                    on_wait=[waits[-1]], on_update=list(si.on_update or [])
                )
            out.append(ins)
        if len(out) != len(bb.instructions):
            bb.instructions = out
    return nc


def _build_score_program():
    """Bass/Tile program: x[CELLS, 100] -> s[CELLS, 2] with
    s[cell, a] = max(x[cell, 10:100]) * x[cell, a].

    Cells are processed partition-major: partition p owns cells
    [p*NT*K, (p+1)*NT*K), split into NT tiles of K cells. All scores
    accumulate into one persistent SBUF tile; a single output DMA at the
    end writes [CELLS, 2] (keeps every DMA at <=1 sync wait — walrus
    rejects DMACopy instructions with 2+ waits)."""
    import concourse.bass as bass
    import concourse.mybir as mybir
    from concourse import tile

    nc = bass.Bass()
    x = nc.declare_dram_parameter("x", [CELLS, C], mybir.dt.float32, isOutput=False)
    s = nc.declare_dram_parameter("s", [CELLS, 2], mybir.dt.float32, isOutput=True)

    # cell = p*(NT*K) + t*K + k
    xv = x.rearrange("(p t k) c -> t p (k c)", p=128, t=NT)     # [NT, 128, K*C]
    sv3 = s.rearrange("(p t ka) a -> t p (ka a)", p=128, t=NT)  # [NT, 128, K*2]

    with tile.TileContext(nc) as tc:
        with (
            tc.tile_pool(name="xin", bufs=NT) as pin,
            tc.tile_pool(name="mx", bufs=2) as pmx,
            tc.tile_pool(name="out", bufs=1) as pout,
        ):
            so = pout.tile([128, NT * K * 2], mybir.dt.float32)
            s4 = so[:].rearrange("p (t k a) -> p t k a", t=NT, a=2)
            for t in range(NT):
                xt = pin.tile([128, K * C], mybir.dt.float32)
                nc.sync.dma_start(xt[:], xv[t])
                x3 = xt[:].rearrange("p (k c) -> p k c", c=C)
                mx = pmx.tile([128, K], mybir.dt.float32)
                nc.vector.reduce_max(
                    out=mx[:], in_=x3[:, :, 10:C], axis=mybir.AxisListType.X
                )
                mxb = mx[:].rearrange("p (k o) -> p k o", o=1).broadcast_to(
                    [128, K, 2]
                )
                nc.vector.tensor_mul(
                    out=s4[:, t, :, :], in0=mxb, in1=x3[:, :, 0:2]
                )
                # per-tile store on the otherwise-idle ACT HWDGE ring, so
                # score stores overlap the input stream without stalling it
                nc.scalar.dma_start(
                    sv3[t], so[:, t * K * 2 : (t + 1) * K * 2]
                )
    return _legalize_multi_waits(nc)


def _build_score_program_raw():
    """Raw-Bass (no Tile) variant: identical dataflow to
    _build_score_program but with hand-rolled semaphores and no Tile
    prologue/epilogue barriers (saves ~15us of fixed overhead).

    SP issues the NT input chains back-to-back on its HWDGE ring; DVE
    consumes tiles in ring order (reduce_max + broadcast mul); ACT
    issues the per-tile score stores on its own HWDGE ring and finally
    waits for all stores to land."""
    import contextlib

    import concourse.bass as bass
    import concourse.mybir as mybir

    nc = bass.Bass()
    x = nc.declare_dram_parameter("x", [CELLS, C], mybir.dt.float32, isOutput=False)
    s = nc.declare_dram_parameter("s", [CELLS, 2], mybir.dt.float32, isOutput=True)

    xv = x.rearrange("(p t k) c -> t p (k c)", p=128, t=NT)     # [NT, 128, K*C]
    sv3 = s.rearrange("(p t ka) a -> t p (ka a)", p=128, t=NT)  # [NT, 128, K*2]

    with contextlib.ExitStack() as ctx:
        xt = [
            ctx.enter_context(
                nc.sbuf_tensor(f"xt{t}", [128, K * C], mybir.dt.float32)
            )
            for t in range(NT)
        ]
        mx = [
            ctx.enter_context(
                nc.sbuf_tensor(f"mx{t}", [128, K], mybir.dt.float32)
            )
            for t in range(NT)
        ]
        so = ctx.enter_context(
            nc.sbuf_tensor("so", [128, NT * K * 2], mybir.dt.float32)
        )
        # One completion semaphore per input chain: a chain's 16 DMA-engine
        # parts each inc +1, and engines process chains at different speeds,
        # so a single shared counter at 16*(t+1) could mix parts of later
        # chains while one engine still owes tile t data (observed as
        # intermittent first-run corruption).
        in_sems = [
            ctx.enter_context(nc.semaphore(f"in_sem{t}")) for t in range(NT)
        ]
        dve_sem = ctx.enter_context(nc.semaphore("dve_sem"))
        out_sem = ctx.enter_context(nc.semaphore("out_sem"))
        block = ctx.enter_context(nc.Block(no_gpsimd_drain=True))

        s4 = so[:].rearrange("p (t k a) -> p t k a", t=NT, a=2)

        @block.sync
        def _(sync):
            for t in range(NT):
                sync.dma_start(xt[t][:], xv[t]).then_inc(in_sems[t], 16)

        @block.vector
        def _(vector):
            for t in range(NT):
                vector.wait_ge(in_sems[t], 16)
                x3 = xt[t][:].rearrange("p (k c) -> p k c", c=C)
                nc.vector.reduce_max(
                    out=mx[t][:], in_=x3[:, :, 10:C], axis=mybir.AxisListType.X
                )
                nc.vector.tensor_mul(
                    out=s4[:, t, :, 0], in0=mx[t][:], in1=x3[:, :, 0]
                )
                nc.vector.tensor_mul(
                    out=s4[:, t, :, 1], in0=mx[t][:], in1=x3[:, :, 1]
                ).then_inc(dve_sem, 1)

        @block.scalar
        def _(scalar):
            for t in range(NT):
                scalar.wait_ge(dve_sem, t + 1)
                scalar.dma_start(
                    sv3[t], so[:, t * K * 2 : (t + 1) * K * 2]
                ).then_inc(out_sem, 16)
            scalar.wait_ge(out_sem, 16 * NT)

    return nc


def _get_program():
    if "nc" not in _CACHE:
        _CACHE["nc"] = _build_score_program_raw()
    return _CACHE["nc"]


def device_scores(xf, trace=False):
    """Run the 8-core score kernel. xf: [G*G, C] f32 contiguous.
    Returns probs [G*G*2] f32 in reference anchor order (n = cell*2 + a),
    plus the BassKernelResults (for profiling when trace=True)."""
    from concourse.bass_utils import run_bass_kernel_spmd

    nc = _get_program()
    in_maps = [
        {"x": xf[i * CELLS : (i + 1) * CELLS]} for i in range(NCORES)
    ]
    res = run_bass_kernel_spmd(
        nc, in_maps, list(range(NCORES)), trace=trace
    )
    # r["s"] is [CELLS, 2] cell-major, so per-core flattening is already
    # the reference anchor order (n_local = cell_local*2 + a).
    probs = np.concatenate(
        [r["s"].reshape(-1) for r in res.results]
    ).astype(np.float32, copy=False)
    return probs, res


def _decode_boxes_for(xf, n):
    """Exact fp32 decode of boxes_xyxy for global anchor indices n."""
    cell = n >> 1
    a = (n & 1).astype(np.int64)
    colf = (cell % G).astype(np.float32)
    rowf = (cell // G).astype(np.float32)
    base = 2 + 4 * a
    cx = (xf[cell, base + 0] + colf) * STRIDE
    cy = (xf[cell, base + 1] + rowf) * STRIDE
    w = np.square(xf[cell, base + 2]) * INPUT_SIZE
    h = np.square(xf[cell, base + 3]) * INPUT_SIZE
    half_w = w / np.float32(2.0)
    half_h = h / np.float32(2.0)
    x1 = cx - half_w
    y1 = cy - half_h
    x2 = cx + half_w - np.float32(1.0)
    y2 = cy + half_h - np.float32(1.0)
    return x1, y1, x2, y2


def _decode_dense(xf, square_wh):
    """Full dense decode (only used when extract_boxes is falsy)."""
    cell = np.arange(G * G)
    colf = (cell % G).astype(np.float32)[:, None]
    rowf = (cell // G).astype(np.float32)[:, None]
    bb = xf[:, 2:10].reshape(G * G, 2, 4)
    cx = (bb[:, :, 0] + colf) * STRIDE
    cy = (bb[:, :, 1] + rowf) * STRIDE
    wh = bb[:, :, 2:4]
    if square_wh:
        wh = np.square(wh)
    wh = wh * INPUT_SIZE
    out = np.concatenate(
        [cx[..., None], cy[..., None], wh], axis=-1
    ).astype(np.float32)
    return out.reshape(1, G, G, 2, 4)


def kernel(inputs, square_wh, extract_boxes, _trace=False):
    x = np.ascontiguousarray(np.asarray(inputs), dtype=np.float32)
    xf = x.reshape(G * G, C)

    if not extract_boxes:
        return _decode_dense(xf, square_wh)

    probs, res = device_scores(xf, trace=_trace)
    sel = _greedy_nms_generic(xf, probs, square_wh)

    valid = sel >= 0
    idx = np.maximum(sel, 0)
    x1, y1, x2, y2 = _decode_boxes_generic(xf, idx, square_wh)
    boxes = np.stack([x1, y1, x2, y2], axis=1).astype(np.float32)
    nms_boxes = np.where(valid[:, None], boxes, np.float32(0.0)).astype(np.float32)
    nms_scores = np.where(valid, probs[idx], np.float32(0.0)).astype(np.float32)
    sel_cells = (idx >> 1)
    cls_sel = np.argmax(xf[sel_cells, 10:C], axis=1).astype(np.int32)
    nms_cls_ids = np.where(valid, cls_sel, np.int32(-1)).astype(np.int32)
    if _trace:
        return (nms_boxes, nms_cls_ids, nms_scores, valid), res
    return nms_boxes, nms_cls_ids, nms_scores, valid


def _decode_boxes_generic(xf, n, square_wh):
    if square_wh:
        return _decode_boxes_for(xf, n)
    cell = n >> 1
    a = (n & 1).astype(np.int64)
    colf = (cell % G).astype(np.float32)
    rowf = (cell // G).astype(np.float32)
    base = 2 + 4 * a
    cx = (xf[cell, base + 0] + colf) * STRIDE
    cy = (xf[cell, base + 1] + rowf) * STRIDE
    w = xf[cell, base + 2] * INPUT_SIZE
    h = xf[cell, base + 3] * INPUT_SIZE
    x1 = cx - w / np.float32(2.0)
    y1 = cy - h / np.float32(2.0)
    x2 = cx + w / np.float32(2.0) - np.float32(1.0)
    y2 = cy + h / np.float32(2.0) - np.float32(1.0)
    return x1, y1, x2, y2


def _greedy_nms_generic(xf, probs, square_wh):
    cand = np.nonzero(probs > SCORE_THR)[0]
    sel = np.full(MAX_OUT, -1, np.int64)
    if cand.size == 0:
        return sel
    cs = probs[cand]
    x1, y1, x2, y2 = _decode_boxes_generic(xf, cand, square_wh)
    area = np.maximum(x2 - x1, np.float32(0.0)) * np.maximum(
        y2 - y1, np.float32(0.0)
    )
    active = np.ones(cand.size, bool)
    neg = np.float32(-1e30)
    for k in range(MAX_OUT):
        masked = np.where(active, cs, neg)
        i = int(np.argmax(masked))
        if not (masked[i] > neg):
            break
        ix1 = np.maximum(x1, x1[i])
        iy1 = np.maximum(y1, y1[i])
        ix2 = np.minimum(x2, x2[i])
        iy2 = np.minimum(y2, y2[i])
        inter = np.maximum(ix2 - ix1, np.float32(0.0)) * np.maximum(
            iy2 - iy1, np.float32(0.0)
        )
        iou = inter / (area + area[i] - inter + np.float32(1e-9))
        active = active & (iou <= IOU_THR)
        sel[k] = cand[i]
    return sel


# revision 50
# speedup vs baseline: 1.0584x; 1.0584x over previous
"""Trainium2 kernel for nn_DecodePredictions (YOLO-style decode + greedy NMS).

Strategy:
  - The memory-bound bulk (reading the [1,512,512,100] f32 input, per-cell
    class-max over 90 classes, per-anchor score = cls_max * objectness) runs
    on 8 NeuronCores, sharded over the 512 grid rows (64 rows per core).
  - Each core streams its 13.1 MB slice through SBUF in 8 big contiguous
    DMA chains (HWDGE line rate) and emits a [32768, 2] f32 score array in
    reference anchor order (cell-major, anchor interleaved).
  - The greedy NMS only ever touches the top ~30 candidates (the reference
    loop locks onto a zero-area box and repeats it), so it runs on host in
    float32 numpy, replicating the reference op-for-op. Device scores are
    bitwise identical to host IEEE fp32, so the end-to-end output matches
    the JAX reference exactly (verified rel err 0.0 across many runs).
"""

import numpy as np

G = 512
C = 100
NCORES = 8
ROWS = G // NCORES            # 64 grid rows per core
CELLS = ROWS * G              # 32768 cells per core
K = 32                        # cells per SBUF partition per tile
NT = CELLS // (128 * K)       # 8 tiles per core
N_CLS = 90

STRIDE = np.float32(16.0)
INPUT_SIZE = np.float32(8192.0)
IOU_THR = np.float32(0.5)
SCORE_THR = np.float32(0.6)
MAX_OUT = 100

_CACHE = {}


def _legalize_multi_waits(nc):
    """This toolchain's walrus rejects any instruction whose sync_info
    carries 2+ waits ("Too many sync wait commands" — the TPB EVENTS slot
    holds exactly one wait). Tile emits multi-wait instructions expecting
    the backend to split them, so do the split here: hoist all but the
    last wait onto single-wait engine NoOps inserted just before the
    instruction (same engine stream => same ordering guarantee)."""
    import concourse.mybir as mybir

    ctr = 0
    for bb in nc.main_func.blocks:
        out = []
        for ins in bb.instructions:
            si = getattr(ins, "sync_info", None)
            waits = list(si.on_wait) if (si is not None and si.on_wait) else []
            if len(waits) > 1:
                for w in waits[:-1]:
                    ctr += 1
                    nop = mybir.InstNoOp(
                        name=f"I-waitsplit-{ctr}", engine=ins.engine
                    )
                    nop.sync_info = mybir.SyncInfo(on_wait=[w], on_update=[])
                    out.append(nop)
                ins.sync_info = mybir.SyncInfo(
                    on_wait=[waits[-1]], on_update=list(si.on_update or [])
                )
            out.append(ins)
        if len(out) != len(bb.instructions):
            bb.instructions = out
    return nc


def _build_score_program():
    """Bass/Tile program: x[CELLS, 100] -> s[CELLS, 2] with
    s[cell, a] = max(x[cell, 10:100]) * x[cell, a].

    Cells are processed partition-major: partition p owns cells
    [p*NT*K, (p+1)*NT*K), split into NT tiles of K cells. All scores
    accumulate into one persistent SBUF tile; a single output DMA at the
    end writes [CELLS, 2] (keeps every DMA at <=1 sync wait — walrus
    rejects DMACopy instructions with 2+ waits)."""
    import concourse.bass as bass
    import concourse.mybir as mybir
    from concourse import tile

    nc = bass.Bass()
    x = nc.declare_dram_parameter("x", [CELLS, C], mybir.dt.float32, isOutput=False)
    s = nc.declare_dram_parameter("s", [CELLS, 2], mybir.dt.float32, isOutput=True)

    # cell = p*(NT*K) + t*K + k
    xv = x.rearrange("(p t k) c -> t p (k c)", p=128, t=NT)     # [NT, 128, K*C]
    sv3 = s.rearrange("(p t ka) a -> t p (ka a)", p=128, t=NT)  # [NT, 128, K*2]

    with tile.TileContext(nc) as tc:
        with (
            tc.tile_pool(name="xin", bufs=NT) as pin,
            tc.tile_pool(name="mx", bufs=2) as pmx,
            tc.tile_pool(name="out", bufs=1) as pout,
        ):
            so = pout.tile([128, NT * K * 2], mybir.dt.float32)
            s4 = so[:].rearrange("p (t k a) -> p t k a", t=NT, a=2)
            for t in range(NT):
                xt = pin.tile([128, K * C], mybir.dt.float32)
                nc.sync.dma_start(xt[:], xv[t])
                x3 = xt[:].rearrange("p (k c) -> p k c", c=C)
                mx = pmx.tile([128, K], mybir.dt.float32)
                nc.vector.reduce_max(
                    out=mx[:], in_=x3[:, :, 10:C], axis=mybir.AxisListType.X
                )
                mxb = mx[:].rearrange("p (k o) -> p k o", o=1).broadcast_to(
                    [128, K, 2]
                )
                nc.vector.tensor_mul(
                    out=s4[:, t, :, :], in0=mxb, in1=x3[:, :, 0:2]
                )
                # per-tile store on the otherwise-idle ACT HWDGE ring, so
                # score stores overlap the input stream without stalling it
                nc.scalar.dma_start(
                    sv3[t], so[:, t * K * 2 : (t + 1) * K * 2]
                )
    return _legalize_multi_waits(nc)


def _build_score_program_raw():
    """Raw-Bass (no Tile) variant: identical dataflow to
    _build_score_program but with hand-rolled semaphores and no Tile
    prologue/epilogue barriers (saves ~15us of fixed overhead).

    SP issues the NT input chains back-to-back on its HWDGE ring; DVE
    consumes tiles in ring order (reduce_max + broadcast mul); ACT
    issues the per-tile score stores on its own HWDGE ring and finally
    waits for all stores to land."""
    import contextlib

    import concourse.bass as bass
    import concourse.mybir as mybir

    nc = bass.Bass()
    x = nc.declare_dram_parameter("x", [CELLS, C], mybir.dt.float32, isOutput=False)
    s = nc.declare_dram_parameter("s", [2, CELLS], mybir.dt.float32, isOutput=True)

    # Non-uniform tiling: full-size tiles for the stream, shrinking tiles at
    # the end so the tail-critical last reduces are short.
    sizes = [K // 2] + [K] * (NT - 1) + [K // 4, K // 4]
    offs = [sum(sizes[:i]) for i in range(len(sizes))]
    ntile = len(sizes)
    ncells_p = NT * K  # cells per partition

    xf2 = x.rearrange("(p n) c -> p (n c)", p=128)         # [128, 256*C]
    sf2 = s.rearrange("a (p n) -> p a n", p=128)           # [128, 2, 256]

    with contextlib.ExitStack() as ctx:
        xt = [
            ctx.enter_context(
                nc.sbuf_tensor(f"xt{t}", [128, sizes[t] * C], mybir.dt.float32)
            )
            for t in range(ntile)
        ]
        mx = [
            ctx.enter_context(
                nc.sbuf_tensor(f"mx{t}", [128, sizes[t]], mybir.dt.float32)
            )
            for t in range(ntile)
        ]
        # Anchor-major score planes packed in one tile: each mul writes a
        # contiguous 128B-aligned per-tile run in its own plane, so the two
        # muls never share a 32B SBUF write granule.
        so2 = ctx.enter_context(
            nc.sbuf_tensor("so2", [128, 2 * NT * K], mybir.dt.float32)
        )
        # One completion semaphore per input chain: a chain's 16 DMA-engine
        # parts each inc +1, and engines process chains at different speeds,
        # so a single shared counter at 16*(t+1) could mix parts of later
        # chains while one engine still owes tile t data (observed as
        # intermittent first-run corruption).
        in_sems = [
            ctx.enter_context(nc.semaphore(f"in_sem{t}")) for t in range(ntile)
        ]
        dve_sem = ctx.enter_context(nc.semaphore("dve_sem"))
        out_sem = ctx.enter_context(nc.semaphore("out_sem"))
        block = ctx.enter_context(nc.Block(no_gpsimd_drain=True))

        @block.sync
        def _(sync):
            for t in range(ntile):
                o, sz = offs[t], sizes[t]
                sync.dma_start(
                    xt[t][:], xf2[:, o * C : (o + sz) * C]
                ).then_inc(in_sems[t], 16)

        mx_pad = ctx.enter_context(
            nc.sbuf_tensor("mx_pad", [128, K], mybir.dt.float32)
        )

        s2v = so2[:].rearrange("p (a n) -> p a n", a=2)  # [128, 2, NT*K]

        def _muls(j):
            """Score muls for tile j. Each carries its own inc; the single
            store chain per tile waits for both."""
            x3j = xt[j][:].rearrange("p (k c) -> p k c", c=C)
            ts = slice(offs[j], offs[j] + sizes[j])
            nc.vector.tensor_mul(
                out=s2v[:, 0, ts], in0=mx[j][:], in1=x3j[:, :, 0]
            ).then_inc(dve_sem, 1)
            nc.vector.tensor_mul(
                out=s2v[:, 1, ts], in0=mx[j][:], in1=x3j[:, :, 1]
            ).then_inc(dve_sem, 1)

        @block.vector
        def _(vector):
            # Software-pipelined one tile deep: tile t's muls are emitted
            # only after tile t+1's reduce. A mul issued back-to-back after
            # the reduce that produced its mx input intermittently reads the
            # reduce's LAST 16B granule before the writeback lands — the
            # DVE does not interlock that same-engine RAW. The reduce
            # between a reduce and its consumers gives the tail writeback
            # ample time.
            for t in range(ntile):
                vector.wait_ge(in_sems[t], 16)
                x3 = xt[t][:].rearrange("p (k c) -> p k c", c=C)
                nc.vector.reduce_max(
                    out=mx[t][:], in_=x3[:, :, 10:C], axis=mybir.AxisListType.X
                )
                if t > 0:
                    _muls(t - 1)
            # spacer before the last tile's muls (narrow re-reduce into a
            # scratch tile, ~0.35us >> granule writeback latency; reads
            # tile 0 so its duration is independent of the last tile size)
            full = sizes.index(K)
            x3f = xt[full][:].rearrange("p (k c) -> p k c", c=C)
            nc.vector.reduce_max(
                out=mx_pad[:, :K],
                in_=x3f[:, :, 10:20],
                axis=mybir.AxisListType.X,
            )
            _muls(ntile - 1)

        # Store groups: one chain per full tile, and ONE chain covering the
        # trailing small tiles (adjacent columns) — each HWDGE dma_start
        # costs ~0.6us of ACT sequencer issue time, and the final chains
        # sit on the kernel's critical tail.
        groups = [(t, t) for t in range(NT - 1)] + [(NT - 1, ntile - 1)]

        @block.scalar
        def _(scalar):
            for t0, t1 in groups:
                ts = slice(offs[t0], offs[t1] + sizes[t1])
                scalar.wait_ge(dve_sem, 2 * (t1 + 1))
                scalar.dma_start(
                    sf2[:, :, ts], s2v[:, :, ts]
                ).then_inc(out_sem, 16)
            scalar.wait_ge(out_sem, 16 * len(groups))

    return nc


def _get_program():
    if "nc" not in _CACHE:
        _CACHE["nc"] = _build_score_program_raw()
    return _CACHE["nc"]


def device_scores(xf, trace=False):
    """Run the 8-core score kernel. xf: [G*G, C] f32 contiguous.
    Returns probs [G*G*2] f32 in reference anchor order (n = cell*2 + a),
    plus the BassKernelResults (for profiling when trace=True)."""
    from concourse.bass_utils import run_bass_kernel_spmd

    nc = _get_program()
    in_maps = [
        {"x": xf[i * CELLS : (i + 1) * CELLS]} for i in range(NCORES)
    ]
    res = run_bass_kernel_spmd(
        nc, in_maps, list(range(NCORES)), trace=trace
    )
    # r["s"] is [2, CELLS] anchor-major; interleave to the reference
    # anchor order (n_local = cell_local*2 + a).
    probs = np.empty(NCORES * CELLS * 2, np.float32)
    for i, r in enumerate(res.results):
        part = probs[i * CELLS * 2 : (i + 1) * CELLS * 2]
        part[0::2] = r["s"][0]
        part[1::2] = r["s"][1]
    return probs, res


def _decode_boxes_for(xf, n):
    """Exact fp32 decode of boxes_xyxy for global anchor indices n."""
    cell = n >> 1
    a = (n & 1).astype(np.int64)
    colf = (cell % G).astype(np.float32)
    rowf = (cell // G).astype(np.float32)
    base = 2 + 4 * a
    cx = (xf[cell, base + 0] + colf) * STRIDE
    cy = (xf[cell, base + 1] + rowf) * STRIDE
    w = np.square(xf[cell, base + 2]) * INPUT_SIZE
    h = np.square(xf[cell, base + 3]) * INPUT_SIZE
    half_w = w / np.float32(2.0)
    half_h = h / np.float32(2.0)
    x1 = cx - half_w
    y1 = cy - half_h
    x2 = cx + half_w - np.float32(1.0)
    y2 = cy + half_h - np.float32(1.0)
    return x1, y1, x2, y2


def _decode_dense(xf, square_wh):
    """Full dense decode (only used when extract_boxes is falsy)."""
    cell = np.arange(G * G)
    colf = (cell % G).astype(np.float32)[:, None]
    rowf = (cell // G).astype(np.float32)[:, None]
    bb = xf[:, 2:10].reshape(G * G, 2, 4)
    cx = (bb[:, :, 0] + colf) * STRIDE
    cy = (bb[:, :, 1] + rowf) * STRIDE
    wh = bb[:, :, 2:4]
    if square_wh:
        wh = np.square(wh)
    wh = wh * INPUT_SIZE
    out = np.concatenate(
        [cx[..., None], cy[..., None], wh], axis=-1
    ).astype(np.float32)
    return out.reshape(1, G, G, 2, 4)


def kernel(inputs, square_wh, extract_boxes, _trace=False):
    x = np.ascontiguousarray(np.asarray(inputs), dtype=np.float32)
    xf = x.reshape(G * G, C)

    if not extract_boxes:
        return _decode_dense(xf, square_wh)

    probs, res = device_scores(xf, trace=_trace)
    sel = _greedy_nms_generic(xf, probs, square_wh)

    valid = sel >= 0
    idx = np.maximum(sel, 0)
    x1, y1, x2, y2 = _decode_boxes_generic(xf, idx, square_wh)
    boxes = np.stack([x1, y1, x2, y2], axis=1).astype(np.float32)
    nms_boxes = np.where(valid[:, None], boxes, np.float32(0.0)).astype(np.float32)
    nms_scores = np.where(valid, probs[idx], np.float32(0.0)).astype(np.float32)
    sel_cells = (idx >> 1)
    cls_sel = np.argmax(xf[sel_cells, 10:C], axis=1).astype(np.int32)
    nms_cls_ids = np.where(valid, cls_sel, np.int32(-1)).astype(np.int32)
    if _trace:
        return (nms_boxes, nms_cls_ids, nms_scores, valid), res
    return nms_boxes, nms_cls_ids, nms_scores, valid


def _decode_boxes_generic(xf, n, square_wh):
    if square_wh:
        return _decode_boxes_for(xf, n)
    cell = n >> 1
    a = (n & 1).astype(np.int64)
    colf = (cell % G).astype(np.float32)
    rowf = (cell // G).astype(np.float32)
    base = 2 + 4 * a
    cx = (xf[cell, base + 0] + colf) * STRIDE
    cy = (xf[cell, base + 1] + rowf) * STRIDE
    w = xf[cell, base + 2] * INPUT_SIZE
    h = xf[cell, base + 3] * INPUT_SIZE
    x1 = cx - w / np.float32(2.0)
    y1 = cy - h / np.float32(2.0)
    x2 = cx + w / np.float32(2.0) - np.float32(1.0)
    y2 = cy + h / np.float32(2.0) - np.float32(1.0)
    return x1, y1, x2, y2


def _greedy_nms_generic(xf, probs, square_wh):
    cand = np.nonzero(probs > SCORE_THR)[0]
    sel = np.full(MAX_OUT, -1, np.int64)
    if cand.size == 0:
        return sel
    cs = probs[cand]
    x1, y1, x2, y2 = _decode_boxes_generic(xf, cand, square_wh)
    area = np.maximum(x2 - x1, np.float32(0.0)) * np.maximum(
        y2 - y1, np.float32(0.0)
    )
    active = np.ones(cand.size, bool)
    neg = np.float32(-1e30)
    for k in range(MAX_OUT):
        masked = np.where(active, cs, neg)
        i = int(np.argmax(masked))
        if not (masked[i] > neg):
            break
        ix1 = np.maximum(x1, x1[i])
        iy1 = np.maximum(y1, y1[i])
        ix2 = np.minimum(x2, x2[i])
        iy2 = np.minimum(y2, y2[i])
        inter = np.maximum(ix2 - ix1, np.float32(0.0)) * np.maximum(
            iy2 - iy1, np.float32(0.0)
        )
        iou = inter / (area + area[i] - inter + np.float32(1e-9))
        active = active & (iou <= IOU_THR)
        sel[k] = cand[i]
    return sel
